# revision 1
# baseline (speedup 1.0000x reference)
"""Trainium2 Bass kernel for nn_FASTMultiHeadAttention (fastmax sparse attention).

Math (per (b,h) pair, n=1024, d=64, t=1):
  qn = q/|q|, kn = k/|k|, v' = v + 0.1*noise
  s_ij = (1 + qn_i.kn_j + qn_i.rpe[n-1+i-j]) * [j<=i]
  out_i = sum_j s_ij v'_j / sum_j s_ij

Device decomposition (T=128 row blocks, j-reversed within blocks):
  - prefix part (full blocks): SS = sum_j [kn|1]^T [v'|1]  (65x65 running PSUM);
    [Q|1]^T.T @ SS gives q.S + colsum(v') in cols 0..63 and q.ksum + count in
    col 64 (the denominator prefix).
  - rpe band: G_I[a,m] = qn_{i0+a}.rpe2[m] (PE) -> fp16 -> skewed DMA write to
    DRAM (row stride SW+1) -> sheared DMA read (row stride SW+2) gives
    B[a,u] = G[a, a+u-127] (u = reversed j), zero guards make the causal edge
    of the diagonal tile exactly 0.
  - per j-tile: PE transpose B slice -> [jrev, a]; diag tile adds (qk+1)*mask;
    SV matmuls: lhsT = scores^T tile, rhs = [v'|1] accumulate out[a, 0..64]
    (col 64 = denominator).
  - out = psum[:, :64] * reciprocal(psum[:, 64]).

Sharding: batch*heads = 64 pairs, 8 per core across 8 NeuronCores.
"""
import sys

sys.path.insert(0, '/opt/trn_rl_repo')

import numpy as np
from contextlib import ExitStack

import concourse.bacc as bacc
import concourse.bass as bass
import concourse.mybir as mybir
import concourse.tile as tile
from concourse.vector_clock import ScopedClock

N_CORES = 8
PPC = 8            # (b,h) pairs per core
N = 1024
D = 64
T = 128
NB = N // T        # 8 row blocks
SW = 1280          # DRAM logical row width for the shear buffer
OFF = 128          # base offset in the shear buffer
GD_SZ = 128 * (SW + 1) + 2048
N_GD = 4           # rotating shear buffers
FP16 = mybir.dt.float16
FP32 = mybir.dt.float32
AL = mybir.AluOpType


class TileContextCompat(tile.TileContext):
    """Walrus in this toolchain rejects >~2 sync waits on one Drain; spread
    the tail-drain waits over single-wait SP nops."""

    def _drain_and_barrier(self, tick_clock, wait_clock):
        nc = self.nc
        drain_inst = nc.sync.drain()
        wait_clock.add_sem_waits(
            drain_inst.ins, ScopedClock({None: tick_clock.global_clock})
        )
        si = drain_inst.ins.sync_info
        waits = list(si.on_wait or [])
        if len(waits) > 1:
            bb = nc.cur_bb.bb
            insts = bb.instructions
            di = insts.index(drain_inst.ins)
            carriers = []
            for w in waits[:-1]:
                nop = nc.sync.nop()
                if nop.ins.sync_info is None:
                    nop.ins.sync_info = mybir.SyncInfo(on_wait=[], on_update=[])
                nop.ins.sync_info.on_wait.append(w)
                carriers.append(nop.ins)
            for c in carriers:
                insts.remove(c)
            for off, c in enumerate(carriers):
                insts.insert(di + off, c)
            si.on_wait.clear()
            si.on_wait.append(waits[-1])

        nc.all_engine_barrier()
        assert self.sems is not None
        popped = nc._tile_sem_poison_stack.pop()
        assert popped is self._sem_poison
        nc.clear_and_free_semaphores(list(self.sems.allocated().values()))
        nc.all_engine_barrier()


def _trace_pair(nc, ctx, tc, pools, consts, gds, p, gd_ctr):
    """Emit instructions for one (b,h) pair index p on this core."""
    qkvn, out = consts['qkvn'], consts['out']
    ident, maskt, rpe2t = consts['ident_t'], consts['maskt_t'], consts['rpe2t_t']
    io, prep, qtp_pool, g16p, brevp, sctp, ss16p, outp = (
        pools['io'], pools['prep'], pools['qt'], pools['g16'], pools['brev'],
        pools['sct'], pools['ss16'], pools['outsb'])
    psg, pst, pso, pss, stat = (
        pools['ps_g'], pools['ps_t'], pools['ps_o'], pools['ps_s'], pools['stat'])

    # ---- load packed inputs [128, 2048] f32: cols 64*(8t+c) : t in q,k,v,nz
    ld = io.tile([128, 4 * 512], FP32, tag="ld")
    src = bass.AP(qkvn, p * (4 * N * D), [[64, 128], [8192, 32], [1, 64]])
    nc.sync.dma_start(ld[:].rearrange("p (tc x) -> p tc x", x=64), src)

    def blk(t, c):
        return ld[:, 64 * (8 * t + c): 64 * (8 * t + c) + 64]

    # ---- prep: normalize q,k -> fp16; vbar; transposes
    qn16 = prep.tile([128, 512], FP16, tag="qn16")
    kbar = prep.tile([128, 520], FP16, tag="kbar")
    vbar = prep.tile([128, 520], FP16, tag="vbar")
    # ones columns (col 64 of each 65-block)
    nc.gpsimd.memset(vbar[:].rearrange("p (b c) -> p b c", c=65)[:, :, 64:65], 1.0)
    nc.gpsimd.memset(kbar[:].rearrange("p (b c) -> p b c", c=65)[:, :, 64:65], 1.0)
    qt = qtp_pool.tile([65, 1024], FP16, tag="qt")
    kt = qtp_pool.tile([64, 1024], FP16, tag="kt")
    nc.gpsimd.memset(qt[64:65, :], 1.0)

    for c in range(NB):
        for (tt, dst) in ((0, qn16[:, 64 * c:64 * c + 64]),
                          (1, kbar[:, 65 * c:65 * c + 64])):
            sq = prep.tile([128, 64], FP16, tag="sq")
            ssq = stat.tile([128, 1], FP32, tag="ssq")
            nc.scalar.activation(sq[:], blk(tt, c),
                                 mybir.ActivationFunctionType.Square,
                                 accum_out=ssq[:])
            nrm = stat.tile([128, 1], FP32, tag="nrm")
            nc.scalar.sqrt(nrm[:], ssq[:])
            rn = stat.tile([128, 1], FP32, tag="rn")
            nc.vector.reciprocal(rn[:], nrm[:])
            nc.scalar.mul(dst, blk(tt, c), rn[:])
        # vbar = 0.1*noise + v
        nc.vector.scalar_tensor_tensor(vbar[:, 65 * c:65 * c + 64], blk(3, c),
                                       0.1, blk(2, c), AL.mult, AL.add)
        # transposes into QT / KT
        qtp = pst.tile([64, 128], FP16, tag="t")
        nc.tensor.transpose(qtp[:], qn16[:, 64 * c:64 * c + 64], ident[:])
        nc.vector.tensor_copy(qt[0:64, 128 * c:128 * c + 128], qtp[:])
        ktp = pst.tile([64, 128], FP16, tag="t")
        nc.tensor.transpose(ktp[:], kbar[:, 65 * c:65 * c + 64], ident[:])
        nc.vector.tensor_copy(kt[:, 128 * c:128 * c + 128], ktp[:])

    ss_ps = pss.tile([65, 65], FP32, tag="ss")
    outsb = outp.tile([128, 512], FP32, tag="o")

    for I in range(NB):
        W = T * (I + 1)
        i0 = T * I
        # ---- G = QT_I.T @ rpe2t[:, :W]  (chunks of 512)
        g16 = g16p.tile([128, 1152], FP16, tag="g16")
        for c0 in range(0, W, 512):
            cw = min(512, W - c0)
            gp = psg.tile([128, 512], FP32, tag="g")
            nc.tensor.matmul(gp[:, :cw], qt[0:64, i0:i0 + 128],
                             rpe2t[:, c0:c0 + cw], start=True, stop=True)
            nc.scalar.copy(g16[:, c0:c0 + cw], gp[:, :cw])
        # ---- shear round trip through DRAM
        gd = gds[gd_ctr[0] % N_GD]
        gd_ctr[0] += 1
        nc.scalar.dma_start(bass.AP(gd, OFF, [[SW + 1, 128], [1, W]]),
                            g16[:, 0:W])
        brev = brevp.tile([128, 1152], FP16, tag="brev")
        nc.sync.dma_start(brev[:, 0:W],
                          bass.AP(gd, OFF - 127, [[SW + 2, 128], [1, W]]))
        # ---- diag qk^T
        qk = pst.tile([128, 128], FP32, tag="t")
        nc.tensor.matmul(qk[:], kt[:, i0:i0 + 128], qt[0:64, i0:i0 + 128],
                         start=True, stop=True)
        # ---- prefix part
        first = True
        if I > 0:
            ss16 = ss16p.tile([65, 65], FP16, tag="ss16")
            nc.scalar.copy(ss16[:], ss_ps[:])
            op = pso.tile([128, 65], FP32, tag="op")
            nc.tensor.matmul(op[:], qt[0:65, i0:i0 + 128], ss16[:],
                             start=True, stop=False)
            first = False
        else:
            op = pso.tile([128, 65], FP32, tag="op")
        # ---- per-J tiles: transpose, assemble, SV
        for J in range(I + 1):
            Dd = T * (I - J)
            bt = pst.tile([128, 128], FP16, tag="t")
            nc.tensor.transpose(bt[:], brev[:, Dd:Dd + 128], ident[:])
            sct = sctp.tile([128, 128], FP16, tag="sct")
            if J == I:
                tmp = sctp.tile([128, 128], FP16, tag="sct")
                nc.vector.scalar_tensor_tensor(tmp[:], qk[:], 1.0, maskt[:],
                                               AL.add, AL.mult)
                nc.vector.tensor_add(sct[:], tmp[:], bt[:])
            else:
                nc.vector.tensor_copy(sct[:], bt[:])
            nc.tensor.matmul(op[:], sct[:], vbar[:, 65 * J:65 * J + 65],
                             start=first, stop=(J == I))
            first = False
        # ---- SS update with block I
        nc.tensor.matmul(ss_ps[:], kbar[:, 65 * I:65 * I + 65],
                         vbar[:, 65 * I:65 * I + 65],
                         start=(I == 0), stop=(I == NB - 1))
        # ---- finalize block
        rd = stat.tile([128, 1], FP32, tag="rd")
        nc.vector.reciprocal(rd[:], op[:, 64:65])
        nc.scalar.mul(outsb[:, 64 * I:64 * I + 64], op[:, 0:64], rd[:])

    # ---- store pair output
    dst = bass.AP(out, p * (N * D), [[64, 128], [8192, 8], [1, 64]])
    nc.sync.dma_start(dst, outsb[:].rearrange("p (c x) -> p c x", x=64))


def build_program(repeat=1):
    nc = bacc.Bacc("TRN2")
    qkvn = nc.dram_tensor("qkvn", [PPC, 4, N, D], FP32, kind="ExternalInput")
    rpe2t = nc.dram_tensor("rpe2t", [D, N], FP16, kind="ExternalInput")
    ident = nc.dram_tensor("ident", [128, 128], FP16, kind="ExternalInput")
    maskt = nc.dram_tensor("maskt", [128, 128], FP16, kind="ExternalInput")
    out = nc.dram_tensor("out", [PPC, N, D], FP32, kind="ExternalOutput")
    gds = [nc.dram_tensor(f"gd{i}", [1, GD_SZ], FP16) for i in range(N_GD)]

    with TileContextCompat(nc) as tc:
        with ExitStack() as ctx:
            pools = {
                'io': ctx.enter_context(tc.tile_pool(name="io", bufs=2)),
                'prep': ctx.enter_context(tc.tile_pool(name="prep", bufs=2)),
                'qt': ctx.enter_context(tc.tile_pool(name="qt", bufs=2)),
                'g16': ctx.enter_context(tc.tile_pool(name="g16", bufs=2)),
                'brev': ctx.enter_context(tc.tile_pool(name="brev", bufs=2)),
                'sct': ctx.enter_context(tc.tile_pool(name="sct", bufs=4)),
                'ss16': ctx.enter_context(tc.tile_pool(name="ss16", bufs=2)),
                'outsb': ctx.enter_context(tc.tile_pool(name="outsb", bufs=2)),
                'stat': ctx.enter_context(tc.tile_pool(name="stat", bufs=8)),
                'const': ctx.enter_context(tc.tile_pool(name="const", bufs=1)),
                'ps_g': ctx.enter_context(
                    tc.tile_pool(name="ps_g", bufs=2, space="PSUM")),
                'ps_t': ctx.enter_context(
                    tc.tile_pool(name="ps_t", bufs=3, space="PSUM")),
                'ps_o': ctx.enter_context(
                    tc.tile_pool(name="ps_o", bufs=2, space="PSUM")),
                'ps_s': ctx.enter_context(
                    tc.tile_pool(name="ps_s", bufs=1, space="PSUM")),
            }
            cp = pools['const']
            ident_t = cp.tile([128, 128], FP16)
            nc.sync.dma_start(ident_t[:], ident[:])
            maskt_t = cp.tile([128, 128], FP16)
            nc.sync.dma_start(maskt_t[:], maskt[:])
            rpe2t_t = cp.tile([64, 1024], FP16)
            nc.sync.dma_start(rpe2t_t[:], rpe2t[:])
            # zero-init shear buffers (guards must be non-NaN; diag guard
            # region must be exactly 0)
            z = cp.tile([128, SW + 1], FP16)
            nc.gpsimd.memset(z[:], 0.0)
            for gd in gds:
                nc.sync.dma_start(
                    bass.AP(gd, 0, [[SW + 1, 128], [1, SW + 1]]), z[:])
            consts = {'qkvn': qkvn, 'out': out, 'ident_t': ident_t,
                      'maskt_t': maskt_t, 'rpe2t_t': rpe2t_t}
            gd_ctr = [0]
            for _ in range(repeat):
                for p in range(PPC):
                    _trace_pair(nc, ctx, tc, pools, consts, gds, p, gd_ctr)
    nc.finalize()
    return nc


_CACHE = {}


def _get_program(repeat=1):
    if repeat not in _CACHE:
        _CACHE[repeat] = build_program(repeat)
    return _CACHE[repeat]


def make_in_maps(q, k, v, drop_noise, rpe_matrix):
    q = np.asarray(q, np.float32).reshape(64, N, D)
    k = np.asarray(k, np.float32).reshape(64, N, D)
    v = np.asarray(v, np.float32).reshape(64, N, D)
    nz = np.asarray(drop_noise, np.float32).reshape(64, N, D)
    rpe = np.asarray(rpe_matrix, np.float32)

    def rev(x):  # reverse rows within each 128-block
        return x.reshape(64, NB, T, D)[:, :, ::-1].reshape(64, N, D)

    kr, vr, nr = rev(k), rev(v), rev(nz)
    rpe2t = np.ascontiguousarray(rpe[N - 1:].T).astype(np.float16)
    ident = np.eye(128, dtype=np.float16)
    r = np.arange(128)
    maskt = ((r[:, None] + r[None, :]) >= 127).astype(np.float16)
    in_maps = []
    for core in range(N_CORES):
        sl = slice(core * PPC, (core + 1) * PPC)
        packed = np.stack([q[sl], kr[sl], vr[sl], nr[sl]], axis=1)
        in_maps.append({
            "qkvn": np.ascontiguousarray(packed),
            "rpe2t": rpe2t, "ident": ident, "maskt": maskt,
        })
    return in_maps


def kernel(q, k, v, drop_noise, rpe_matrix):
    from concourse import bass2jax
    nc = _get_program(1)
    in_maps = make_in_maps(q, k, v, drop_noise, rpe_matrix)
    results = bass2jax.run_bass_via_pjrt(nc, in_maps, n_cores=N_CORES)
    outs = np.stack([results[c]["out"] for c in range(N_CORES)])
    return outs.reshape(4, 16, N, D).astype(np.float32)
